# revision 26
# baseline (speedup 1.0000x reference)
"""MultiHeadedAttention Trainium2 kernel (8 NeuronCores).

Sharding: core c -> (batch b = c//2, head-group g = c%2). Each core computes
the 8-head attention slice for one batch plus its partial output projection;
the host sums the two partials per batch and adds the output bias.

Device-side layout is feature-major: the host ships q/k/v pre-transposed
([E, L], bf16) so every matmul contracts along SBUF partitions without any
on-chip transpose. Projection biases are folded into the PSUM->SBUF
evacuation (per-partition tensor_scalar add for q/k, broadcast tensor_tensor
add for v); the attention scale 1/sqrt(hd) is folded into Wq/bq on the host.

Attention math per head (no max-subtraction; scores are ~N(0,1) so exp is
safe): S^T = kh^T.T @ qh^T in PSUM, exp on ScalarE -> bf16 SBUF, then
O^T/sums in one PSUM accumulation using a ones-augmented V ([L, 65] lhsT),
normalization via approx-reciprocal + gpsimd partition_broadcast, and the
output projection consumes O^T directly as lhsT.

The attention phase is software-pipelined: steps = (head-pair, 1024-wide
q-block); each step's 32 granules emit the S matmuls + exp for this step
interleaved with the PV matmuls + normalization of the previous step, plus
leftover projection work (v and qk pd-tiles 1..3) so the TensorEngine stays
dense enough to keep the HAM clock-gate warm. Each granule's S psum tile
[128, 1024] holds both heads of the pair side by side so the two K=64
matmuls issue back-to-back into different row-groups/banks (concurrent).
"""

import math
import sys

sys.path.insert(0, "/opt/trn_rl_repo")

import numpy as np
import ml_dtypes

import concourse.bass as bass  # noqa: F401  (registers rust bindings)
import concourse.mybir as mybir
import concourse.tile as tile
from concourse import bacc
from concourse.bass_utils import run_bass_kernel_spmd

BF16_NP = ml_dtypes.bfloat16
F32 = mybir.dt.float32
BF16 = mybir.dt.bfloat16

B, L, E, H, HD = 4, 2048, 1024, 16, 64
NCORES = 8
D = 512          # per-core projection width (8 heads * 64)
HL = 8           # local heads per core
P = 128
ET = E // P      # 8 contraction tiles over E
PT = D // P      # 4 partition-tiles of qpT/kpT
TT = L // P      # 16 token tiles
NBLK = 2         # qtok blocks of 1024
QW = 512         # matmul moving free width

TRACE = False
USE_APPROX_RECIP = False
LAST_EXEC_NS = None
LAST_RESULTS = None


def _emit(nc, tc, io):
    Exp = mybir.ActivationFunctionType.Exp
    qT, kT, vT, wq, wk, wv, wo, bq, bk, bv, out = (
        io["qT"], io["kT"], io["vT"], io["wq"], io["wk"], io["wv"],
        io["wo"], io["bq"], io["bk"], io["bv"], io["out"],
    )

    import contextlib
    stack = contextlib.ExitStack()
    with stack:
        pers = stack.enter_context(tc.tile_pool(name="pers", bufs=1))
        outp = stack.enter_context(tc.tile_pool(name="outp", bufs=2))
        inx = stack.enter_context(tc.tile_pool(name="inx", bufs=16))
        expS = stack.enter_context(tc.tile_pool(name="expS", bufs=36))
        ps_pool = stack.enter_context(
            tc.tile_pool(name="ps", bufs=3, space="PSUM"))
        pv_pool = stack.enter_context(
            tc.tile_pool(name="pv", bufs=2, space="PSUM"))
        sc_pool = stack.enter_context(tc.tile_pool(name="sc", bufs=4))
        bc_pool = stack.enter_context(tc.tile_pool(name="bc", bufs=1))

        wo_sb = [pers.tile([P, E], BF16, tag=f"wo{c}", name=f"wo{c}")
                 for c in range(4)]
        qpT = [pers.tile([P, L], BF16, tag=f"qpT{i}", name=f"qpT{i}")
               for i in range(PT)]
        kpT = [pers.tile([P, L], BF16, tag=f"kpT{i}", name=f"kpT{i}")
               for i in range(PT)]
        vpa = [pers.tile([P, HL * (HD + 1)], BF16, tag=f"vpa{t}",
                         name=f"vpa{t}") for t in range(TT)]
        OT = [pers.tile([P, L], BF16, tag=f"OT{i}", name=f"OT{i}")
              for i in range(PT)]
        w_sb = {}
        for nm in ("q", "k", "v"):
            w_sb[nm] = [pers.tile([P, D], BF16, tag=f"w{nm}{e}",
                                  name=f"w{nm}{e}") for e in range(ET)]
        bq_sb = pers.tile([P, PT], F32, tag="bq", name="bq_sb")
        bk_sb = pers.tile([P, PT], F32, tag="bk", name="bk_sb")
        bv_row = pers.tile([1, D], F32, tag="bvr", name="bv_row")
        bv_bc = pers.tile([P, D], F32, tag="bvb", name="bv_bc")
        dummy = pers.tile([1, 8], F32, tag="dm", name="dummy")
        b_sb = {"q": bq_sb, "k": bk_sb}

        # prime the exp table load early so the first real exp is cheap
        nc.vector.memset(dummy[:], 0.0)
        nc.scalar.activation(dummy[:], dummy[:], Exp)
        nc.sync.dma_start(out=bq_sb[:], in_=bq[:, :])
        nc.sync.dma_start(out=bk_sb[:], in_=bk[:, :])
        nc.sync.dma_start(out=bv_row[:], in_=bv[:, :])



        def load_weights(nm, wdram):
            for e in range(ET):
                nc.sync.dma_start(out=w_sb[nm][e][:],
                                  in_=wdram[P * e:P * (e + 1), :])

        def late_loads():
            # gpsimd (SWDGE) so these don't clog the Sync issue queue
            for e in range(ET):
                nc.gpsimd.dma_start(out=w_sb["v"][e][:],
                                    in_=wv[P * e:P * (e + 1), :])
            for c in range(4):
                nc.gpsimd.dma_start(out=wo_sb[c][:],
                                    in_=wo[P * c:P * (c + 1), :])
            for t in range(TT):
                nc.vector.memset(vpa[t][:], 1.0)
            nc.gpsimd.partition_broadcast(bv_bc[:], bv_row[:], channels=P)

        # ---- projection building blocks ----
        def dma_quarter(xT, qu):
            # alternate Sync/GpSimd DGE paths: issue bandwidth, not HBM
            # bandwidth, is the scarce resource for these 128KB loads
            xt = [inx.tile([P, QW], BF16, tag="inx", name="inx")
                  for _ in range(ET)]
            for e in range(ET):
                eng = nc.sync if e % 2 == 0 else nc.gpsimd
                eng.dma_start(
                    out=xt[e][:],
                    in_=xT[P * e:P * (e + 1), QW * qu:QW * (qu + 1)])
            return xt

        def qk_group(nm, dst, xt, qu, i):
            """One psum group: qpT/kpT pd-tile i, token quarter qu."""
            ps = ps_pool.tile([P, QW], F32, tag="s", name="ps")
            for e in range(ET):
                nc.tensor.matmul(ps[:], w_sb[nm][e][:, P * i:P * (i + 1)],
                                 xt[e][:], start=(e == 0), stop=(e == ET - 1))
            nc.vector.tensor_scalar_add(
                dst[i][:, QW * qu:QW * (qu + 1)], ps[:],
                b_sb[nm][:, i:i + 1])

        def v_group(xt, qu, tt_):
            t = 4 * qu + tt_
            ps = ps_pool.tile([P, D], F32, tag="s", name="ps")
            for e in range(ET):
                nc.tensor.matmul(ps[:], xt[e][:, P * tt_:P * (tt_ + 1)],
                                 w_sb["v"][e][:], start=(e == 0),
                                 stop=(e == ET - 1))
            src3 = ps.rearrange("p (h x) -> p h x", x=HD)
            bv3 = bv_bc.rearrange("p (h x) -> p h x", x=HD)
            dst3 = vpa[t].rearrange("p (h x) -> p h x", x=HD + 1)[:, :, 0:HD]
            nc.vector.tensor_add(dst3, src3, bv3)

        # Interleaved projection task lists, one per early attention step.
        # Each task is a closure emitting one psum group (~9 matmuls).
        def make_tasks():
            tasks = {si: [] for si in range(4)}
            # step 0 first finishes qpT[0]/kpT[0] quarters 2-3 (needed from
            # step (0, blk1) onward), then runs the whole v projection
            for qu in (2, 3):
                for nm, xTd, dst in (("q", qT, qpT), ("k", kT, kpT)):
                    def dma_x0(qu=qu, xTd=xTd):
                        return dma_quarter(xTd, qu)
                    tasks[0].append(("dma", dma_x0))
                    tasks[0].append(
                        ("grp", lambda xt, nm=nm, dst=dst, qu=qu:
                            qk_group(nm, dst, xt, qu, 0)))
            # the v projection (PV of pair 0 starts at step 1)
            for qu in range(4):
                def dma_v(qu=qu):
                    return dma_quarter(vT, qu)
                tasks[0].append(("dma", dma_v))
                for tt_ in range(4):
                    tasks[0].append(
                        ("grp", lambda xt, qu=qu, tt_=tt_: v_group(xt, qu, tt_)))
            # q & k pd-tile i is first used at step 2*i, so pd1 rides step 1
            # and pd2/pd3 spread over steps 2-3 / 4-5: steps 4-7 are
            # otherwise scalar-bound, so late proj work there is free
            sched = {1: [(1, (0, 1, 2, 3))],
                     2: [(2, (0, 1))], 3: [(2, (2, 3))],
                     4: [(3, (0, 1))], 5: [(3, (2, 3))]}
            for si, specs in sched.items():
                tasks.setdefault(si, [])
                for i, qus in specs:
                    for nm, xTd, dst in (("q", qT, qpT), ("k", kT, kpT)):
                        for qu in qus:
                            def dma_x(qu=qu, xTd=xTd):
                                return dma_quarter(xTd, qu)
                            tasks[si].append(("dma", dma_x))
                            tasks[si].append(
                                ("grp", lambda xt, nm=nm, dst=dst, qu=qu,
                                    i=i: qk_group(nm, dst, xt, qu, i)))
            return tasks

        proj_tasks = make_tasks()

        def run_task(state, task):
            kind, fn = task
            if kind == "dma":
                state["xt"] = fn()
            else:
                fn(state["xt"])

        # ---- pre-step: q & k pd-tile 0, token quarters 0-1 only ----
        # (step (0, blk0) reads quarters 0-1 of qpT[0]/kpT[0]; quarters 2-3
        # are first needed at step (0, blk1) and ride the step-0 tasks)
        load_weights("q", wq)
        xq = dma_quarter(qT, 0)
        load_weights("k", wk)
        xk = dma_quarter(kT, 0)
        qk_group("q", qpT, xq, 0, 0)
        qk_group("k", kpT, xk, 0, 0)
        late_loads()
        xq = dma_quarter(qT, 1)
        xk = dma_quarter(kT, 1)
        qk_group("q", qpT, xq, 1, 0)
        qk_group("k", kpT, xk, 1, 0)

        def outproj_tile(t):
            ost = outp.tile([P, E], BF16, tag="outst", name="outst")
            for n in range(2):
                ps = ps_pool.tile([P, QW], F32, tag="s", name="ps")
                for c in range(4):
                    nc.tensor.matmul(
                        ps[:], OT[c][:, P * t:P * (t + 1)],
                        wo_sb[c][:, QW * n:QW * (n + 1)],
                        start=(c == 0), stop=(c == 3))
                nc.vector.tensor_copy(ost[:, QW * n:QW * (n + 1)], ps[:])
                eng = nc.sync if n == 0 else nc.gpsimd
                eng.dma_start(
                    out=out[P * t:P * (t + 1), QW * n:QW * (n + 1)],
                    in_=ost[:, QW * n:QW * (n + 1)])

        # ---- attention steps ----
        def emit_norm_piece(pend, gi):
            """Normalization for the step PV'd last iteration, spread over
            granules 2..9 of the current iteration (keeps big DVE ops away
            from the step boundary so they don't block projection casts)."""
            if pend is None:
                return
            if 2 <= gi <= 5:
                sc = pend["sc"][gi - 2]
                nc.vector.reciprocal_approx_fast(sc[:], sc[:])
            elif 6 <= gi <= 9:
                g = gi - 6
                php2, pblk2 = pend["step"]
                jj, hh = g // 2, g % 2
                qt = 2 * pblk2 + jj
                bc = bc_pool.tile([P, QW], F32, tag="bc", name="bc")
                nc.gpsimd.partition_broadcast(
                    bc[:], pend["sc"][g][:], channels=P)
                sl = OT[php2][64 * hh:64 * (hh + 1), QW * qt:QW * (qt + 1)]
                nc.vector.tensor_mul(sl, sl, bc[64 * hh:64 * (hh + 1), :])

        steps = [(hp, blk) for hp in range(4) for blk in range(NBLK)]
        saved = {}
        prev = None
        norm_pending = None
        for idx in range(len(steps) + 1):
            cur = steps[idx] if idx < len(steps) else None
            if cur is not None:
                saved[cur] = [[None, None] for _ in range(TT)]
            tasks = proj_tasks.pop(idx, [])
            tstate = {}
            ntask = len(tasks)
            pv_live = {}
            sc_step = [None] * 4
            for gi in range(2 * TT):        # 32 granules per step
                emit_norm_piece(norm_pending, gi)
                if gi == 10:
                    norm_pending = None
                if cur is not None:
                    hp, blk = cur
                    kt, j = gi // 2, gi % 2
                    q0 = 1024 * blk + QW * j
                    ps = ps_pool.tile([P, 1024], F32, tag="s", name="ps")
                    for half in range(2):
                        nc.tensor.matmul(
                            ps[:, QW * half:QW * (half + 1)],
                            kpT[hp][64 * half:64 * (half + 1),
                                    P * kt:P * (kt + 1)],
                            qpT[hp][64 * half:64 * (half + 1), q0:q0 + QW],
                            start=True, stop=True)
                    e = expS.tile([P, 1024], BF16, tag="expS", name="expS")
                    nc.scalar.activation(e[:], ps[:], Exp)
                    saved[cur][kt][j] = e
                if prev is not None and (cur is not None or gi < TT):
                    php, pblk = prev
                    # granules 0-15: groups (hh=0,jj=0) & (hh=1,jj=0)
                    # granules 16-31: jj=1; within each phase alternate hh.
                    # Drain step (cur None): 4 matmuls/granule over 16
                    # granules so norms and outproj can start earlier.
                    if cur is None:
                        jj = gi // 8
                        nk = 4
                        k0 = (gi % 8) // 2 * 4
                    else:
                        jj = gi // TT
                        nk = 2
                        k0 = (gi % TT) // 2 * 2
                    hh = gi % 2
                    lh = 2 * php + hh
                    key = (hh, jj)
                    if key not in pv_live:
                        pv_live[key] = pv_pool.tile([HD + 1, QW], F32,
                                                    tag="pv", name="pv")
                    pv = pv_live[key]
                    for kk in range(k0, k0 + nk):
                        nc.tensor.matmul(
                            pv[:],
                            vpa[kk][:, (HD + 1) * lh:(HD + 1) * lh + HD + 1],
                            saved[prev][kk][jj][:, QW * hh:QW * (hh + 1)],
                            start=(kk == 0), stop=(kk == TT - 1))
                    if k0 + nk == TT:
                        # group finished: stage O^T (unnormalized) + sums row
                        qt = 2 * pblk + jj
                        row = 2 * jj + hh
                        nc.vector.tensor_copy(
                            OT[php][64 * hh:64 * (hh + 1),
                                    QW * qt:QW * (qt + 1)],
                            pv[0:HD, :])
                        sc = sc_pool.tile([1, QW], F32, tag="sc",
                                          name="sc")
                        nc.vector.tensor_copy(sc[:], pv[HD:HD + 1, :])
                        sc_step[row] = sc
                        del pv_live[key]
                        if cur is None:
                            # drain step: normalize inline to shorten the
                            # tail before the output projection
                            nc.vector.reciprocal_approx_fast(sc[:], sc[:])
                            bc = bc_pool.tile([P, QW], F32, tag="bc",
                                              name="bc")
                            nc.gpsimd.partition_broadcast(bc[:], sc[:],
                                                          channels=P)
                            sl = OT[php][64 * hh:64 * (hh + 1),
                                         QW * qt:QW * (qt + 1)]
                            nc.vector.tensor_mul(
                                sl, sl, bc[64 * hh:64 * (hh + 1), :])
                if cur is None:
                    # drain step: overlap the output projection with the
                    # final PV/norm work as OT columns become ready
                    # (pair-3 qt0/1 norms emitted by granule 9, qt2 by 7,
                    # qt3 by 15 with the compressed PV schedule)
                    if 10 <= gi < 18:
                        outproj_tile(gi - 10)
                    elif 18 <= gi < 22:
                        outproj_tile(gi - 18 + 8)
                    elif 22 <= gi < 26:
                        outproj_tile(gi - 22 + 12)
                # interleaved projection tasks, spread over the step
                if ntask:
                    t0 = ntask * gi // (2 * TT)
                    t1 = ntask * (gi + 1) // (2 * TT)
                    for ti in range(t0, t1):
                        run_task(tstate, tasks[ti])
            if prev is not None and cur is not None:
                norm_pending = {"step": prev, "sc": sc_step}
            prev = cur
            if idx >= 2:
                saved.pop(steps[idx - 2], None)




def build_nc():
    nc = bacc.Bacc("TRN2", target_bir_lowering=False, debug=False,
                   num_devices=NCORES)
    io = {
        "qT": nc.dram_tensor("qT", [E, L], BF16, kind="ExternalInput").ap(),
        "kT": nc.dram_tensor("kT", [E, L], BF16, kind="ExternalInput").ap(),
        "vT": nc.dram_tensor("vT", [E, L], BF16, kind="ExternalInput").ap(),
        "wq": nc.dram_tensor("wq", [E, D], BF16, kind="ExternalInput").ap(),
        "wk": nc.dram_tensor("wk", [E, D], BF16, kind="ExternalInput").ap(),
        "wv": nc.dram_tensor("wv", [E, D], BF16, kind="ExternalInput").ap(),
        "wo": nc.dram_tensor("wo", [D, E], BF16, kind="ExternalInput").ap(),
        "bq": nc.dram_tensor("bq", [P, PT], F32, kind="ExternalInput").ap(),
        "bk": nc.dram_tensor("bk", [P, PT], F32, kind="ExternalInput").ap(),
        "bv": nc.dram_tensor("bv", [1, D], F32, kind="ExternalInput").ap(),
        "out": nc.dram_tensor("out", [L, E], BF16,
                              kind="ExternalOutput").ap(),
    }
    with tile.TileContext(nc) as tc:
        _emit(nc, tc, io)
    nc.compile()
    return nc


_NC = None


def _get_nc():
    global _NC
    if _NC is None:
        _NC = build_nc()
    return _NC


def make_in_maps(q, k, v, Wq, bq, Wk, bk, Wv, bv, Wo):
    scale = np.float32(1.0 / math.sqrt(HD))
    in_maps = []
    for c in range(NCORES):
        b, g = divmod(c, 2)
        sl = slice(g * D, (g + 1) * D)
        in_maps.append({
            "qT": np.ascontiguousarray(q[b].T).astype(BF16_NP),
            "kT": np.ascontiguousarray(k[b].T).astype(BF16_NP),
            "vT": np.ascontiguousarray(v[b].T).astype(BF16_NP),
            "wq": (Wq[:, sl] * scale).astype(BF16_NP),
            "wk": np.ascontiguousarray(Wk[:, sl]).astype(BF16_NP),
            "wv": np.ascontiguousarray(Wv[:, sl]).astype(BF16_NP),
            "wo": np.ascontiguousarray(Wo[sl, :]).astype(BF16_NP),
            "bq": np.ascontiguousarray(
                (bq[sl] * scale).reshape(PT, P).T).astype(np.float32),
            "bk": np.ascontiguousarray(
                bk[sl].reshape(PT, P).T).astype(np.float32),
            "bv": bv[sl].reshape(1, D).astype(np.float32),
        })
    return in_maps


def kernel(q, k, v, mask, Wq, bq, Wk, bk, Wv, bv, Wo, bo):
    global LAST_EXEC_NS, LAST_RESULTS
    q, k, v = (np.asarray(x, np.float32) for x in (q, k, v))
    Wq, bq, Wk, bk, Wv, bv, Wo, bo = (
        np.asarray(x, np.float32)
        for x in (Wq, bq, Wk, bk, Wv, bv, Wo, bo))
    nc = _get_nc()
    in_maps = make_in_maps(q, k, v, Wq, bq, Wk, bk, Wv, bv, Wo)
    kwargs = {}
    if TRACE:
        kwargs = dict(trace=True)
    res = run_bass_kernel_spmd(nc, in_maps, list(range(NCORES)), **kwargs)
    LAST_EXEC_NS = res.exec_time_ns
    LAST_RESULTS = res
    outs = [np.asarray(res.results[c]["out"], np.float32)
            for c in range(NCORES)]
    full = np.stack([outs[2 * b] + outs[2 * b + 1] for b in range(B)], axis=0)
    full += bo[None, None, :].astype(np.float32)
    return full.astype(np.float32)



# revision 27
# speedup vs baseline: 1.1673x; 1.1673x over previous
"""MultiHeadedAttention Trainium2 kernel (8 NeuronCores).

Sharding: core c -> (batch b = c//2, head-group g = c%2). Each core computes
the 8-head attention slice for one batch plus its partial output projection;
the host sums the two partials per batch and adds the output bias.

Device-side layout is feature-major: the host ships q/k/v pre-transposed
([E, L], bf16) so every matmul contracts along SBUF partitions without any
on-chip transpose. Projection biases are folded into the PSUM->SBUF
evacuation (per-partition tensor_scalar add for q/k, broadcast tensor_tensor
add for v); the attention scale 1/sqrt(hd) is folded into Wq/bq on the host.

Attention math per head (no max-subtraction; scores are ~N(0,1) so exp is
safe): S^T = kh^T.T @ qh^T in PSUM, exp on ScalarE -> bf16 SBUF, then
O^T/sums in one PSUM accumulation using a ones-augmented V ([L, 65] lhsT),
normalization via approx-reciprocal + gpsimd partition_broadcast, and the
output projection consumes O^T directly as lhsT.

The attention phase is software-pipelined: steps = (head-pair, 1024-wide
q-block); each step's 32 granules emit the S matmuls + exp for this step
interleaved with the PV matmuls + normalization of the previous step, plus
leftover projection work (v and qk pd-tiles 1..3) so the TensorEngine stays
dense enough to keep the HAM clock-gate warm. Each granule's S psum tile
[128, 1024] holds both heads of the pair side by side so the two K=64
matmuls issue back-to-back into different row-groups/banks (concurrent).
"""

import math
import sys

sys.path.insert(0, "/opt/trn_rl_repo")

import numpy as np
import ml_dtypes

import concourse.bass as bass  # noqa: F401  (registers rust bindings)
import concourse.mybir as mybir
import concourse.tile as tile
from concourse import bacc
from concourse.bass_utils import run_bass_kernel_spmd

BF16_NP = ml_dtypes.bfloat16
F32 = mybir.dt.float32
BF16 = mybir.dt.bfloat16

B, L, E, H, HD = 4, 2048, 1024, 16, 64
NCORES = 8
D = 512          # per-core projection width (8 heads * 64)
HL = 8           # local heads per core
P = 128
ET = E // P      # 8 contraction tiles over E
PT = D // P      # 4 partition-tiles of qpT/kpT
TT = L // P      # 16 token tiles
NBLK = 2         # qtok blocks of 1024
QW = 512         # matmul moving free width

TRACE = False
USE_APPROX_RECIP = False
LAST_EXEC_NS = None
LAST_RESULTS = None


def _emit(nc, tc, io):
    Exp = mybir.ActivationFunctionType.Exp
    qT, kT, vT, wq, wk, wv, wo, bq, bk, bv, out = (
        io["qT"], io["kT"], io["vT"], io["wq"], io["wk"], io["wv"],
        io["wo"], io["bq"], io["bk"], io["bv"], io["out"],
    )

    import contextlib
    stack = contextlib.ExitStack()
    with stack:
        pers = stack.enter_context(tc.tile_pool(name="pers", bufs=1))
        outp = stack.enter_context(tc.tile_pool(name="outp", bufs=2))
        inx = stack.enter_context(tc.tile_pool(name="inx", bufs=16))
        expS = stack.enter_context(tc.tile_pool(name="expS", bufs=36))
        ps_pool = stack.enter_context(
            tc.tile_pool(name="ps", bufs=3, space="PSUM"))
        pv_pool = stack.enter_context(
            tc.tile_pool(name="pv", bufs=2, space="PSUM"))
        sc_pool = stack.enter_context(tc.tile_pool(name="sc", bufs=4))
        bc_pool = stack.enter_context(tc.tile_pool(name="bc", bufs=1))

        wo_sb = [pers.tile([P, E], BF16, tag=f"wo{c}", name=f"wo{c}")
                 for c in range(4)]
        qpT = [pers.tile([P, L], BF16, tag=f"qpT{i}", name=f"qpT{i}")
               for i in range(PT)]
        kpT = [pers.tile([P, L], BF16, tag=f"kpT{i}", name=f"kpT{i}")
               for i in range(PT)]
        vpa = [pers.tile([P, HL * (HD + 1)], BF16, tag=f"vpa{t}",
                         name=f"vpa{t}") for t in range(TT)]
        OT = [pers.tile([P, L], BF16, tag=f"OT{i}", name=f"OT{i}")
              for i in range(PT)]
        w_sb = {}
        for nm in ("q", "k", "v"):
            w_sb[nm] = [pers.tile([P, D], BF16, tag=f"w{nm}{e}",
                                  name=f"w{nm}{e}") for e in range(ET)]
        bq_sb = pers.tile([P, PT], F32, tag="bq", name="bq_sb")
        bk_sb = pers.tile([P, PT], F32, tag="bk", name="bk_sb")
        bv_row = pers.tile([1, D], F32, tag="bvr", name="bv_row")
        bv_bc = pers.tile([P, D], F32, tag="bvb", name="bv_bc")
        dummy = pers.tile([1, 8], F32, tag="dm", name="dummy")
        b_sb = {"q": bq_sb, "k": bk_sb}

        # prime the exp table load early so the first real exp is cheap
        nc.vector.memset(dummy[:], 0.0)
        nc.scalar.activation(dummy[:], dummy[:], Exp)
        nc.sync.dma_start(out=bq_sb[:], in_=bq[:, :])
        nc.sync.dma_start(out=bk_sb[:], in_=bk[:, :])
        nc.sync.dma_start(out=bv_row[:], in_=bv[:, :])

        # warm the PE HAM clock-gate during the initial weight/input DMA
        # wait so the first real projection groups run at full clock
        nc.vector.memset(vpa[0][:], 1.0)
        nc.vector.memset(vpa[1][:], 1.0)
        warm = ps_pool.tile([P, QW], F32, tag="s", name="warm")
        for w in range(20):
            nc.tensor.matmul(warm[:], vpa[0][:, 0:P], vpa[1][:, 0:QW],
                             start=(w == 0), stop=(w == 19))



        def load_weights(nm, wdram):
            for e in range(ET):
                nc.sync.dma_start(out=w_sb[nm][e][:],
                                  in_=wdram[P * e:P * (e + 1), :])

        def late_loads():
            # gpsimd (SWDGE) so these don't clog the Sync issue queue
            for e in range(ET):
                nc.gpsimd.dma_start(out=w_sb["v"][e][:],
                                    in_=wv[P * e:P * (e + 1), :])
            for c in range(4):
                nc.gpsimd.dma_start(out=wo_sb[c][:],
                                    in_=wo[P * c:P * (c + 1), :])
            for t in range(TT):
                nc.vector.memset(vpa[t][:], 1.0)
            nc.gpsimd.partition_broadcast(bv_bc[:], bv_row[:], channels=P)

        # ---- projection building blocks ----
        def dma_quarter(xT, qu):
            # alternate Sync/GpSimd DGE paths: issue bandwidth, not HBM
            # bandwidth, is the scarce resource for these 128KB loads
            xt = [inx.tile([P, QW], BF16, tag="inx", name="inx")
                  for _ in range(ET)]
            for e in range(ET):
                eng = nc.sync if e % 2 == 0 else nc.gpsimd
                eng.dma_start(
                    out=xt[e][:],
                    in_=xT[P * e:P * (e + 1), QW * qu:QW * (qu + 1)])
            return xt

        def qk_group(nm, dst, xt, qu, i):
            """One psum group: qpT/kpT pd-tile i, token quarter qu."""
            ps = ps_pool.tile([P, QW], F32, tag="s", name="ps")
            for e in range(ET):
                nc.tensor.matmul(ps[:], w_sb[nm][e][:, P * i:P * (i + 1)],
                                 xt[e][:], start=(e == 0), stop=(e == ET - 1))
            nc.vector.tensor_scalar_add(
                dst[i][:, QW * qu:QW * (qu + 1)], ps[:],
                b_sb[nm][:, i:i + 1])

        def v_group(xt, qu, tt_):
            t = 4 * qu + tt_
            ps = ps_pool.tile([P, D], F32, tag="s", name="ps")
            for e in range(ET):
                nc.tensor.matmul(ps[:], xt[e][:, P * tt_:P * (tt_ + 1)],
                                 w_sb["v"][e][:], start=(e == 0),
                                 stop=(e == ET - 1))
            src3 = ps.rearrange("p (h x) -> p h x", x=HD)
            bv3 = bv_bc.rearrange("p (h x) -> p h x", x=HD)
            dst3 = vpa[t].rearrange("p (h x) -> p h x", x=HD + 1)[:, :, 0:HD]
            nc.vector.tensor_add(dst3, src3, bv3)

        # Interleaved projection task lists, one per early attention step.
        # Each task is a closure emitting one psum group (~9 matmuls).
        def make_tasks():
            tasks = {si: [] for si in range(4)}
            # step 0 first finishes qpT[0]/kpT[0] quarters 2-3 (needed from
            # step (0, blk1) onward), then runs the whole v projection
            for qu in (2, 3):
                for nm, xTd, dst in (("q", qT, qpT), ("k", kT, kpT)):
                    def dma_x0(qu=qu, xTd=xTd):
                        return dma_quarter(xTd, qu)
                    tasks[0].append(("dma", dma_x0))
                    tasks[0].append(
                        ("grp", lambda xt, nm=nm, dst=dst, qu=qu:
                            qk_group(nm, dst, xt, qu, 0)))
            # the v projection (PV of pair 0 starts at step 1)
            for qu in range(4):
                def dma_v(qu=qu):
                    return dma_quarter(vT, qu)
                tasks[0].append(("dma", dma_v))
                for tt_ in range(4):
                    tasks[0].append(
                        ("grp", lambda xt, qu=qu, tt_=tt_: v_group(xt, qu, tt_)))
            # q & k pd-tile i is first used at step 2*i, so pd1 rides step 1
            # and pd2/pd3 spread over steps 2-3 / 4-5: steps 4-7 are
            # otherwise scalar-bound, so late proj work there is free
            sched = {1: [(1, (0, 1, 2, 3))],
                     2: [(2, (0, 1))], 3: [(2, (2, 3))],
                     4: [(3, (0, 1))], 5: [(3, (2, 3))]}
            for si, specs in sched.items():
                tasks.setdefault(si, [])
                for i, qus in specs:
                    for nm, xTd, dst in (("q", qT, qpT), ("k", kT, kpT)):
                        for qu in qus:
                            def dma_x(qu=qu, xTd=xTd):
                                return dma_quarter(xTd, qu)
                            tasks[si].append(("dma", dma_x))
                            tasks[si].append(
                                ("grp", lambda xt, nm=nm, dst=dst, qu=qu,
                                    i=i: qk_group(nm, dst, xt, qu, i)))
            return tasks

        proj_tasks = make_tasks()

        def run_task(state, task):
            kind, fn = task
            if kind == "dma":
                state["xt"] = fn()
            else:
                fn(state["xt"])

        # ---- pre-step: q & k pd-tile 0, token quarters 0-1 only ----
        # (step (0, blk0) reads quarters 0-1 of qpT[0]/kpT[0]; quarters 2-3
        # are first needed at step (0, blk1) and ride the step-0 tasks)
        load_weights("q", wq)
        xq = dma_quarter(qT, 0)
        load_weights("k", wk)
        xk = dma_quarter(kT, 0)
        qk_group("q", qpT, xq, 0, 0)
        qk_group("k", kpT, xk, 0, 0)
        late_loads()
        xq = dma_quarter(qT, 1)
        xk = dma_quarter(kT, 1)
        qk_group("q", qpT, xq, 1, 0)
        qk_group("k", kpT, xk, 1, 0)

        def outproj_tile(t):
            ost = outp.tile([P, E], BF16, tag="outst", name="outst")
            for n in range(2):
                ps = ps_pool.tile([P, QW], F32, tag="s", name="ps")
                for c in range(4):
                    nc.tensor.matmul(
                        ps[:], OT[c][:, P * t:P * (t + 1)],
                        wo_sb[c][:, QW * n:QW * (n + 1)],
                        start=(c == 0), stop=(c == 3))
                nc.vector.tensor_copy(ost[:, QW * n:QW * (n + 1)], ps[:])
                eng = nc.sync if n == 0 else nc.gpsimd
                eng.dma_start(
                    out=out[P * t:P * (t + 1), QW * n:QW * (n + 1)],
                    in_=ost[:, QW * n:QW * (n + 1)])

        # ---- attention steps ----
        def emit_norm_piece(pend, gi):
            """Normalization for the step PV'd last iteration, spread over
            granules 2..9 of the current iteration (keeps big DVE ops away
            from the step boundary so they don't block projection casts)."""
            if pend is None:
                return
            if 2 <= gi <= 5:
                sc = pend["sc"][gi - 2]
                nc.vector.reciprocal_approx_fast(sc[:], sc[:])
            elif 6 <= gi <= 9:
                g = gi - 6
                php2, pblk2 = pend["step"]
                jj, hh = g // 2, g % 2
                qt = 2 * pblk2 + jj
                bc = bc_pool.tile([P, QW], F32, tag="bc", name="bc")
                nc.gpsimd.partition_broadcast(
                    bc[:], pend["sc"][g][:], channels=P)
                sl = OT[php2][64 * hh:64 * (hh + 1), QW * qt:QW * (qt + 1)]
                nc.vector.tensor_mul(sl, sl, bc[64 * hh:64 * (hh + 1), :])

        steps = [(hp, blk) for hp in range(4) for blk in range(NBLK)]
        saved = {}
        prev = None
        norm_pending = None
        for idx in range(len(steps) + 1):
            cur = steps[idx] if idx < len(steps) else None
            if cur is not None:
                saved[cur] = [[None, None] for _ in range(TT)]
            tasks = proj_tasks.pop(idx, [])
            tstate = {}
            ntask = len(tasks)
            pv_live = {}
            sc_step = [None] * 4
            for gi in range(2 * TT):        # 32 granules per step
                emit_norm_piece(norm_pending, gi)
                if gi == 10:
                    norm_pending = None
                if cur is not None:
                    hp, blk = cur
                    kt, j = gi // 2, gi % 2
                    q0 = 1024 * blk + QW * j
                    ps = ps_pool.tile([P, 1024], F32, tag="s", name="ps")
                    for half in range(2):
                        nc.tensor.matmul(
                            ps[:, QW * half:QW * (half + 1)],
                            kpT[hp][64 * half:64 * (half + 1),
                                    P * kt:P * (kt + 1)],
                            qpT[hp][64 * half:64 * (half + 1), q0:q0 + QW],
                            start=True, stop=True)
                    e = expS.tile([P, 1024], BF16, tag="expS", name="expS")
                    nc.scalar.activation(e[:], ps[:], Exp)
                    saved[cur][kt][j] = e
                if prev is not None and (cur is not None or gi < TT):
                    php, pblk = prev
                    # granules 0-15: groups (hh=0,jj=0) & (hh=1,jj=0)
                    # granules 16-31: jj=1; within each phase alternate hh.
                    # Drain step (cur None): 4 matmuls/granule over 16
                    # granules so norms and outproj can start earlier.
                    if cur is None:
                        jj = gi // 8
                        nk = 4
                        k0 = (gi % 8) // 2 * 4
                    else:
                        jj = gi // TT
                        nk = 2
                        k0 = (gi % TT) // 2 * 2
                    hh = gi % 2
                    lh = 2 * php + hh
                    key = (hh, jj)
                    if key not in pv_live:
                        pv_live[key] = pv_pool.tile([HD + 1, QW], F32,
                                                    tag="pv", name="pv")
                    pv = pv_live[key]
                    for kk in range(k0, k0 + nk):
                        nc.tensor.matmul(
                            pv[:],
                            vpa[kk][:, (HD + 1) * lh:(HD + 1) * lh + HD + 1],
                            saved[prev][kk][jj][:, QW * hh:QW * (hh + 1)],
                            start=(kk == 0), stop=(kk == TT - 1))
                    if k0 + nk == TT:
                        # group finished: stage O^T (unnormalized) + sums row
                        qt = 2 * pblk + jj
                        row = 2 * jj + hh
                        nc.vector.tensor_copy(
                            OT[php][64 * hh:64 * (hh + 1),
                                    QW * qt:QW * (qt + 1)],
                            pv[0:HD, :])
                        sc = sc_pool.tile([1, QW], F32, tag="sc",
                                          name="sc")
                        nc.vector.tensor_copy(sc[:], pv[HD:HD + 1, :])
                        sc_step[row] = sc
                        del pv_live[key]
                        if cur is None:
                            # drain step: normalize inline to shorten the
                            # tail before the output projection
                            nc.vector.reciprocal_approx_fast(sc[:], sc[:])
                            bc = bc_pool.tile([P, QW], F32, tag="bc",
                                              name="bc")
                            nc.gpsimd.partition_broadcast(bc[:], sc[:],
                                                          channels=P)
                            sl = OT[php][64 * hh:64 * (hh + 1),
                                         QW * qt:QW * (qt + 1)]
                            nc.vector.tensor_mul(
                                sl, sl, bc[64 * hh:64 * (hh + 1), :])
                if cur is None:
                    # drain step: overlap the output projection with the
                    # final PV/norm work as OT columns become ready
                    # (pair-3 qt0/1 norms emitted by granule 9, qt2 by 7,
                    # qt3 by 15 with the compressed PV schedule)
                    if 10 <= gi < 18:
                        outproj_tile(gi - 10)
                    elif 18 <= gi < 22:
                        outproj_tile(gi - 18 + 8)
                    elif 22 <= gi < 26:
                        outproj_tile(gi - 22 + 12)
                # interleaved projection tasks, spread over the step
                if ntask:
                    t0 = ntask * gi // (2 * TT)
                    t1 = ntask * (gi + 1) // (2 * TT)
                    for ti in range(t0, t1):
                        run_task(tstate, tasks[ti])
            if prev is not None and cur is not None:
                norm_pending = {"step": prev, "sc": sc_step}
            prev = cur
            if idx >= 2:
                saved.pop(steps[idx - 2], None)




def build_nc():
    nc = bacc.Bacc("TRN2", target_bir_lowering=False, debug=False,
                   num_devices=NCORES)
    io = {
        "qT": nc.dram_tensor("qT", [E, L], BF16, kind="ExternalInput").ap(),
        "kT": nc.dram_tensor("kT", [E, L], BF16, kind="ExternalInput").ap(),
        "vT": nc.dram_tensor("vT", [E, L], BF16, kind="ExternalInput").ap(),
        "wq": nc.dram_tensor("wq", [E, D], BF16, kind="ExternalInput").ap(),
        "wk": nc.dram_tensor("wk", [E, D], BF16, kind="ExternalInput").ap(),
        "wv": nc.dram_tensor("wv", [E, D], BF16, kind="ExternalInput").ap(),
        "wo": nc.dram_tensor("wo", [D, E], BF16, kind="ExternalInput").ap(),
        "bq": nc.dram_tensor("bq", [P, PT], F32, kind="ExternalInput").ap(),
        "bk": nc.dram_tensor("bk", [P, PT], F32, kind="ExternalInput").ap(),
        "bv": nc.dram_tensor("bv", [1, D], F32, kind="ExternalInput").ap(),
        "out": nc.dram_tensor("out", [L, E], BF16,
                              kind="ExternalOutput").ap(),
    }
    with tile.TileContext(nc) as tc:
        _emit(nc, tc, io)
    nc.compile()
    return nc


_NC = None


def _get_nc():
    global _NC
    if _NC is None:
        _NC = build_nc()
    return _NC


def make_in_maps(q, k, v, Wq, bq, Wk, bk, Wv, bv, Wo):
    scale = np.float32(1.0 / math.sqrt(HD))
    in_maps = []
    for c in range(NCORES):
        b, g = divmod(c, 2)
        sl = slice(g * D, (g + 1) * D)
        in_maps.append({
            "qT": np.ascontiguousarray(q[b].T).astype(BF16_NP),
            "kT": np.ascontiguousarray(k[b].T).astype(BF16_NP),
            "vT": np.ascontiguousarray(v[b].T).astype(BF16_NP),
            "wq": (Wq[:, sl] * scale).astype(BF16_NP),
            "wk": np.ascontiguousarray(Wk[:, sl]).astype(BF16_NP),
            "wv": np.ascontiguousarray(Wv[:, sl]).astype(BF16_NP),
            "wo": np.ascontiguousarray(Wo[sl, :]).astype(BF16_NP),
            "bq": np.ascontiguousarray(
                (bq[sl] * scale).reshape(PT, P).T).astype(np.float32),
            "bk": np.ascontiguousarray(
                bk[sl].reshape(PT, P).T).astype(np.float32),
            "bv": bv[sl].reshape(1, D).astype(np.float32),
        })
    return in_maps


def kernel(q, k, v, mask, Wq, bq, Wk, bk, Wv, bv, Wo, bo):
    global LAST_EXEC_NS, LAST_RESULTS
    q, k, v = (np.asarray(x, np.float32) for x in (q, k, v))
    Wq, bq, Wk, bk, Wv, bv, Wo, bo = (
        np.asarray(x, np.float32)
        for x in (Wq, bq, Wk, bk, Wv, bv, Wo, bo))
    nc = _get_nc()
    in_maps = make_in_maps(q, k, v, Wq, bq, Wk, bk, Wv, bv, Wo)
    kwargs = {}
    if TRACE:
        kwargs = dict(trace=True)
    res = run_bass_kernel_spmd(nc, in_maps, list(range(NCORES)), **kwargs)
    LAST_EXEC_NS = res.exec_time_ns
    LAST_RESULTS = res
    outs = [np.asarray(res.results[c]["out"], np.float32)
            for c in range(NCORES)]
    full = np.stack([outs[2 * b] + outs[2 * b + 1] for b in range(B)], axis=0)
    full += bo[None, None, :].astype(np.float32)
    return full.astype(np.float32)

